# revision 13
# baseline (speedup 1.0000x reference)
"""Trainium2 Bass kernel for AttentionSocialPooling.

Strategy (8 cores, data parallel over batch B=8; core m handles batch b=m):
For each (b,t) the N x N pairwise attention MLP is decomposed as
  hidden[i,j,a] = relu(u[i,a] + v[j,a]),  u = pos@(W1p-W1d)+b1, v = pos@W1d
(all scaled by |W2[a]|, channels permuted so positive-W2 channels come first).
H[j, (i,a)] is materialized by one PE matmul: lhsT rows = [1; 1; v_hi; v_lo]
(bf16 hi/lo split for ~fp32 accuracy), rhs rows = [u_hi; u_lo; delta; delta]
where delta is a constant block-identity pattern.  relu on ACT engine (fp16
out); the signed channel reduction on DVE (two strided reduces); the
subtraction, radius mask and att*mask products on GPSIMD; sigmoid on ACT
batched over 4 timesteps; dist^2 via a second fp16 matmul; final row sums via
PE matmuls with w^T / mask^T as the stationary operand accumulating into one
persistent PSUM bank, postprocessed once at the end of the kernel.
"""

import numpy as np
import ml_dtypes

B, T, N, C, A = 8, 64, 128, 2, 16
R2 = 2500.0

bf16 = ml_dtypes.bfloat16

_CACHE = {}


def _host_prep(positions, W1, b1, W2, b2):
    pos = np.asarray(positions, dtype=np.float32)
    W1 = np.asarray(W1, dtype=np.float32)
    b1 = np.asarray(b1, dtype=np.float32)
    W2 = np.asarray(W2, dtype=np.float32)
    b2 = np.asarray(b2, dtype=np.float32)

    W1p, W1d = W1[:C], W1[C:]
    w2 = W2[:, 0]
    pos_idx = np.where(w2 >= 0)[0]
    neg_idx = np.where(w2 < 0)[0]
    np2, nn2 = len(pos_idx), len(neg_idx)
    A2 = np2 + nn2

    # permuted + |W2|-scaled channel coefficient matrices
    Wu2 = np.zeros((C, A2), np.float32)
    Wd2 = np.zeros((C, A2), np.float32)
    b1v = np.zeros((A2,), np.float32)
    for k, a in enumerate(pos_idx):
        g = abs(w2[a])
        Wu2[:, k] = g * (W1p[:, a] - W1d[:, a])
        Wd2[:, k] = g * W1d[:, a]
        b1v[k] = g * b1[a]
    for k, a in enumerate(neg_idx):
        g = abs(w2[a])
        Wu2[:, np2 + k] = g * (W1p[:, a] - W1d[:, a])
        Wd2[:, np2 + k] = g * W1d[:, a]
        b1v[np2 + k] = g * b1[a]

    u = pos @ Wu2 + b1v          # [B,T,N,A2]
    v = pos @ Wd2                # [B,T,N,A2]

    uhi = u.astype(bf16)
    ulo = (u - uhi.astype(np.float32)).astype(bf16)
    vhi = v.astype(bf16)
    vlo = (v - vhi.astype(np.float32)).astype(bf16)

    # lhsT for the H matmul: rows [1; 1; v_hi(A2); v_lo(A2)] (per core)
    vT = np.empty((B, 2 * A2 + 2, T * N), dtype=bf16)
    vT[:, 0:2] = np.asarray(1.0, dtype=bf16)
    vT[:, 2:A2 + 2] = vhi.transpose(0, 3, 1, 2).reshape(B, A2, T * N)
    vT[:, A2 + 2:] = vlo.transpose(0, 3, 1, 2).reshape(B, A2, T * N)

    # per-t rhs rows for u: [T, 2, N*A2] bf16 (per core)
    uflat = np.empty((B, T, 2, N * A2), dtype=bf16)
    uflat[:, :, 0] = uhi.reshape(B, T, N * A2)
    uflat[:, :, 1] = ulo.reshape(B, T, N * A2)

    # block-identity delta pattern, stacked twice (hi+lo rows) [2*A2, N*A2]
    delta1 = np.zeros((A2, N * A2), dtype=bf16)
    for a in range(A2):
        delta1[a, a::A2] = np.asarray(1.0, dtype=bf16)
    delta = np.concatenate([delta1, delta1], axis=0)

    # dist^2 matmul operands, fp16 hi/lo split (K=10, cross terms kept)
    f16 = np.float16
    pos64 = pos.astype(np.float64)
    n2 = (pos64 ** 2).sum(-1)        # [B,T,N] (float64)
    px = pos64[..., 0].reshape(B, T * N)
    py = pos64[..., 1].reshape(B, T * N)
    n2f = n2.reshape(B, T * N)

    def hilo(x):
        hi = x.astype(f16)
        lo = (x - hi.astype(np.float64)).astype(f16)
        return hi, lo

    pxh, pxl = hilo(px)
    pyh, pyl = hilo(py)
    n2h, n2l = hilo(n2f)
    m2pxh, m2pxl = hilo(-2 * px)
    m2pyh, m2pyl = hilo(-2 * py)
    ones = np.ones_like(pxh)
    lhsTd = np.stack([pxh, pxh, pxl, pyh, pyh, pyl, ones, ones, n2h, n2l],
                     axis=1).astype(f16)                     # [B,10,T*N]
    rhsd = np.stack([m2pxh, m2pxl, m2pxh, m2pyh, m2pyl, m2pyh, n2h, n2l,
                     ones, ones], axis=1).astype(f16)

    # final-matmul rhs, fp16 hi/lo: per t 6 cols (pxh,pyh,1, pxl,pyl,0)
    pos3 = np.empty((B, N, T * 6), f16)
    p6 = pos3.reshape(B, N, T, 6)
    p6[..., 0] = pxh.reshape(B, T, N).transpose(0, 2, 1)
    p6[..., 1] = pyh.reshape(B, T, N).transpose(0, 2, 1)
    p6[..., 2] = 1.0
    p6[..., 3] = pxl.reshape(B, T, N).transpose(0, 2, 1)
    p6[..., 4] = pyl.reshape(B, T, N).transpose(0, 2, 1)
    p6[..., 5] = 0.0

    offd1 = (1.0 - np.eye(N)).astype(f16)
    offd = np.concatenate([offd1, offd1], axis=1)   # [N, 2N]

    return dict(vT=vT, uflat=uflat, delta=delta, lhsTd=lhsTd, rhsd=rhsd,
                pos3=pos3, offd=offd, A2=A2, np2=np2, nn2=nn2,
                b2=float(b2[0]))


def _build_program(A2, np2, nn2, b2val):
    import concourse.bacc as bacc
    import concourse.mybir as mybir
    import concourse.tile as tile

    f32 = mybir.dt.float32
    f16 = mybir.dt.float16
    bfl = mybir.dt.bfloat16
    Alu = mybir.AluOpType
    Act = mybir.ActivationFunctionType
    X = mybir.AxisListType.X

    K2 = 2 * A2 + 2
    NA = N * A2
    HALF = 64 * A2          # columns per PSUM half-tile

    nc = bacc.Bacc()

    vT_p = nc.declare_dram_parameter("vT", [K2, T * N], bfl, isOutput=False)
    uflat_p = nc.declare_dram_parameter("uflat", [T, 2, NA], bfl, isOutput=False)
    lhsTd_p = nc.declare_dram_parameter("lhsTd", [10, T * N], f16, isOutput=False)
    rhsd_p = nc.declare_dram_parameter("rhsd", [10, T * N], f16, isOutput=False)
    delta_p = nc.declare_dram_parameter("delta", [2 * A2, NA], bfl, isOutput=False)
    pos3_p = nc.declare_dram_parameter("pos3", [N, T * 6], f16, isOutput=False)
    offd_p = nc.declare_dram_parameter("offd", [N, 2 * N], f16, isOutput=False)
    out_p = nc.declare_dram_parameter("out", [T, N, C], f32, isOutput=True)

    with tile.TileContext(nc) as tc:
        with (
            tc.tile_pool(name="pers", bufs=1) as pers,
            tc.tile_pool(name="hpsum", bufs=2, space="PSUM") as hpsum,
            tc.tile_pool(name="dpsum", bufs=2, space="PSUM") as dpsum,
            tc.tile_pool(name="fpsum", bufs=1, space="PSUM") as fpsum,
            tc.tile_pool(name="work", bufs=3) as work,
            tc.tile_pool(name="wsmall", bufs=3) as wsmall,
        ):
            vT_s = pers.tile([K2, T * N], bfl, tag="vT")
            lhsTd_s = pers.tile([10, T * N], f16, tag="lhsTd")
            rhsd_s = pers.tile([10, T * N], f16, tag="rhsd")
            pos3_s = pers.tile([N, T * 6], f16, tag="pos3")
            offd_s = pers.tile([N, 2 * N], f16, tag="offd")
            rhsH = [pers.tile([K2, NA], bfl, tag=f"rhsH{i}", name=f"rhsH{i}")
                    for i in range(4)]

            nc.gpsimd.dma_start(vT_s[:], vT_p[:])
            nc.gpsimd.dma_start(lhsTd_s[:], lhsTd_p[:])
            nc.gpsimd.dma_start(rhsd_s[:], rhsd_p[:])
            nc.gpsimd.dma_start(pos3_s[:], pos3_p[:])
            nc.gpsimd.dma_start(offd_s[:], offd_p[:])
            for i in range(4):
                nc.gpsimd.dma_start(rhsH[i][2:2 * A2 + 2, :], delta_p[:])

            # persistent PSUM bank for the final row-sum matmuls: 8 cols/t
            # cols per t: [w@pxh, w@pyh, sum_w, w@pxl, w@pyl, 0, cnt, pad]
            pf = fpsum.tile([N, 8 * T], f32, tag="F")

            chunks = []
            off = 0
            while off < HALF:
                cn = min(512, HALF - off)
                chunks.append((off, cn))
                off += cn

            attP2 = attM2 = m12 = None
            mask2 = [None, None]     # the two 2t-pair masks of a 4t group
            w4 = pre4 = att4 = None
            pd2 = [None, None]

            for t in range(T):
                rh = rhsH[t % 4]
                nc.sync.dma_start(rh[0:2, :], uflat_p[t])

                Rt = work.tile([N, NA], f16, tag="R")
                for h in range(2):
                    ph = hpsum.tile([N, HALF], f32, tag="H")
                    for (off, cn) in chunks:
                        nc.tensor.matmul(
                            ph[:, off:off + cn],
                            vT_s[:, t * N:(t + 1) * N],
                            rh[:, h * HALF + off:h * HALF + off + cn],
                            start=True, stop=True,
                        )
                    if h == 0 and t % 4 == 1:
                        # rebalance: one of every 8 relu half-tiles runs on
                        # the (now lighter-loaded) DVE instead of ACT
                        nc.vector.tensor_scalar_max(
                            Rt[:, h * HALF:(h + 1) * HALF], ph[:], 0.0)
                    else:
                        nc.scalar.activation(Rt[:, h * HALF:(h + 1) * HALF],
                                             ph[:], Act.Relu)

                R3 = Rt[:].rearrange("p (i a) -> p i a", a=A2)
                g2 = t % 2
                g4 = t % 4
                q = g4 // 2          # which 2t-pair inside the 4t group
                if g2 == 0:
                    mask2[q] = wsmall.tile([N, 2 * N], f16, tag="mask", name="mask2")
                    pd2[q] = dpsum.tile([N, 2 * N], f32, tag="D", name="pd2")
                if g4 == 0:
                    pre4 = wsmall.tile([N, 4 * N], f16, tag="pre")
                    att4 = wsmall.tile([N, 4 * N], f16, tag="att")
                    w4 = wsmall.tile([N, 4 * N], f16, tag="w")

                # signed channel reduction as a fold tree: pairing one
                # positive with one negative channel makes level 1 a
                # tensor_tensor subtract, which (unlike tensor_reduce) runs
                # in the 2x fp16 DVE mode.  A=16 so after pairing the slot
                # count is always 8 -> three clean fold levels.
                preT = pre4[:, g4 * N:(g4 + 1) * N]
                if np2 and nn2:
                    m = min(np2, nn2)
                    L = A2 - 2 * m
                    T8 = work.tile([N, 8 * N], f16, tag="T8")
                    T3 = T8[:].rearrange("p (i s) -> p i s", s=8)
                    nc.vector.tensor_sub(T3[:, :, 0:m], R3[:, :, 0:m],
                                         R3[:, :, np2:np2 + m])
                    if L:
                        base = m if np2 > nn2 else np2 + m
                        if np2 > nn2:
                            nc.vector.tensor_add(
                                T3[:, :, m:8], R3[:, :, base:base + L:2],
                                R3[:, :, base + 1:base + L:2])
                        else:
                            tmpL = work.tile([N, (L // 2) * N], f16, tag="tmpL")
                            t3l = tmpL[:].rearrange("p (i s) -> p i s", s=L // 2)
                            nc.vector.tensor_add(
                                t3l[:, :, :], R3[:, :, base:base + L:2],
                                R3[:, :, base + 1:base + L:2])
                            nc.vector.tensor_scalar_mul(T3[:, :, m:8],
                                                        t3l[:, :, :], -1.0)
                    U4 = work.tile([N, 4 * N], f16, tag="U4")
                    U4v = U4[:].rearrange("p (i s) -> p i s", s=4)
                    nc.vector.tensor_add(U4v[:], T3[:, :, 0:4], T3[:, :, 4:8])
                    U2 = work.tile([N, 2 * N], f16, tag="U2")
                    U2v = U2[:].rearrange("p (i s) -> p i s", s=2)
                    nc.vector.tensor_add(U2v[:], U4v[:, :, 0:2], U4v[:, :, 2:4])
                    nc.vector.tensor_add(preT, U2v[:, :, 0], U2v[:, :, 1])
                else:
                    sgn = 1.0 if np2 else -1.0
                    tmpR = work.tile([N, N], f32, tag="tmpR")
                    nc.vector.tensor_reduce(tmpR[:], R3[:, :, 0:A2], axis=X,
                                            op=Alu.add)
                    nc.vector.tensor_scalar_mul(preT, tmpR[:], sgn)

                nc.tensor.matmul(pd2[q][:, g2 * N:(g2 + 1) * N],
                                 lhsTd_s[:, t * N:(t + 1) * N],
                                 rhsd_s[:, t * N:(t + 1) * N],
                                 start=True, stop=True)

                if g2 == 1:
                    # mask = (d^2 < R2) * offd.  The diagonal is excluded by
                    # offd; d^2 > 0 holds for every other pair (up to pairs at
                    # ~zero distance whose contribution is ~0 anyway), so the
                    # lower-bound test of the reference is not needed.  One
                    # DVE op (GPSIMD cannot read PSUM).
                    nc.vector.scalar_tensor_tensor(mask2[q][:], pd2[q][:], R2,
                                                   offd_s[:],
                                                   op0=Alu.is_lt, op1=Alu.mult)

                if g4 == 3:
                    nc.scalar.activation(att4[:], pre4[:], Act.Sigmoid,
                                         bias=b2val, scale=1.0)
                    nc.gpsimd.tensor_mul(w4[:, 0:2 * N], att4[:, 0:2 * N],
                                         mask2[0][:])
                    nc.gpsimd.tensor_mul(w4[:, 2 * N:4 * N], att4[:, 2 * N:4 * N],
                                         mask2[1][:])
                    for dt_ in range(4):
                        tt = t - 3 + dt_
                        s = dt_ * N
                        nc.tensor.matmul(pf[:, 8 * tt:8 * tt + 6],
                                         w4[:, s:s + N],
                                         pos3_s[:, 6 * tt:6 * tt + 6],
                                         start=True, stop=True)
                        nc.tensor.matmul(pf[:, 8 * tt + 6:8 * tt + 7],
                                         mask2[dt_ // 2][:, (dt_ % 2) * N:(dt_ % 2 + 1) * N],
                                         pos3_s[:, 6 * tt + 2:6 * tt + 3],
                                         start=True, stop=True)

            # ---- tail: one pass over all T timesteps ----
            # pf viewed as [p, t, 8]; pos3 viewed as [p, t, 6]
            pf3 = pf[:].rearrange("p (t c) -> p t c", c=8)
            p3 = pos3_s[:].rearrange("p (t c) -> p t c", c=6)
            cntT = work.tile([N, T], f32, tag="cntT")
            rcpT = work.tile([N, T], f32, tag="rcpT")
            swT = work.tile([N, 2 * T], f32, tag="swT")
            outst = work.tile([N, 2 * T], f32, tag="outst")
            nc.vector.tensor_scalar_max(cntT[:], pf3[:, :, 6], 1e-6)
            nc.vector.reciprocal(rcpT[:], cntT[:])
            s3 = swT[:].rearrange("p (c t) -> p c t", c=2)
            o3 = outst[:].rearrange("p (t c) -> p t c", c=2)
            for c in range(2):
                # pos_i reconstructed as hi+lo (one PSUM read per op)
                nc.vector.tensor_add(s3[:, c], p3[:, :, c], p3[:, :, c + 3])
                nc.vector.tensor_mul(s3[:, c], pf3[:, :, 2], s3[:, c])
                nc.vector.tensor_sub(o3[:, :, c], pf3[:, :, c], s3[:, c])
                nc.vector.tensor_add(o3[:, :, c], o3[:, :, c], pf3[:, :, c + 3])
                nc.vector.tensor_mul(o3[:, :, c], o3[:, :, c], rcpT[:])
            nc.sync.dma_start(out_p[:].rearrange("t n c -> n t c"), outst[:])

    nc.compile()
    return nc


def kernel(positions, W1, b1, W2, b2, _trace=False, _trace_kwargs=None):
    from concourse.bass_utils import run_bass_kernel_spmd

    prep = _host_prep(positions, W1, b1, W2, b2)
    A2, np2, nn2, b2v = prep["A2"], prep["np2"], prep["nn2"], prep["b2"]

    key = (A2, np2, nn2, b2v)
    if key not in _CACHE:
        _CACHE[key] = _build_program(A2, np2, nn2, b2v)
    nc = _CACHE[key]

    in_maps = []
    for b in range(B):
        in_maps.append({
            "vT": np.ascontiguousarray(prep["vT"][b]),
            "uflat": np.ascontiguousarray(prep["uflat"][b]),
            "delta": prep["delta"],
            "lhsTd": np.ascontiguousarray(prep["lhsTd"][b]),
            "rhsd": np.ascontiguousarray(prep["rhsd"][b]),
            "pos3": np.ascontiguousarray(prep["pos3"][b]),
            "offd": prep["offd"],
        })

    kw = {}
    if _trace:
        kw["trace"] = True
        if _trace_kwargs:
            kw.update(_trace_kwargs)
    res = run_bass_kernel_spmd(nc, in_maps, list(range(B)), **kw)
    out = np.stack([r["out"] for r in res.results], axis=0).astype(np.float32)
    if _trace:
        return out, res
    return out


# revision 15
# speedup vs baseline: 1.0428x; 1.0428x over previous
"""Trainium2 Bass kernel for AttentionSocialPooling.

Strategy (8 cores, data parallel over batch B=8; core m handles batch b=m):
For each (b,t) the N x N pairwise attention MLP is decomposed as
  hidden[i,j,a] = relu(u[i,a] + v[j,a]),  u = pos@(W1p-W1d)+b1, v = pos@W1d
(channels scaled by |W2[a]|/2 and permuted so positive-W2 channels come
first; column order is channel-blocked: all positive-channel columns
(i-major) then all negative ones, so the two signed reductions read
contiguous memory).  H[j, cols] comes from one PE matmul in fp8e4m3
DoubleRow mode (4-way hi/lo cascade of u and v for ~fp16 accuracy at 2
columns/cycle).  relu on ACT (fp16 out); signed channel reduction = two DVE
tensor_reduces; pre-subtraction and att*mask on GPSIMD; sigmoid on ACT
(scale=2 undoes the 1/2 prescale) batched over 4 timesteps; dist^2 via a
small fp16 matmul; mask = (d^2 < R^2)*offdiag as one DVE op; final row sums
via PE matmuls with w^T / mask^T stationary accumulating into one persistent
PSUM bank, postprocessed once at the end.
"""

import numpy as np
import ml_dtypes

B, T, N, C, A = 8, 64, 128, 2, 16
R2 = 2500.0

bf16 = ml_dtypes.bfloat16
f8 = ml_dtypes.float8_e4m3fn

_CACHE = {}


def _f8_cascade(x, levels=4):
    """Split x into `levels` fp8e4m3 terms summing to ~x."""
    terms = []
    r = x.astype(np.float32)
    for _ in range(levels):
        h = r.astype(f8)
        terms.append(h)
        r = r - h.astype(np.float32)
    return terms


def _host_prep(positions, W1, b1, W2, b2):
    pos = np.asarray(positions, dtype=np.float32)
    W1 = np.asarray(W1, dtype=np.float32)
    b1 = np.asarray(b1, dtype=np.float32)
    W2 = np.asarray(W2, dtype=np.float32)
    b2 = np.asarray(b2, dtype=np.float32)

    W1p, W1d = W1[:C], W1[C:]
    w2 = W2[:, 0]
    pos_idx = np.where(w2 >= 0)[0]
    neg_idx = np.where(w2 < 0)[0]
    np2, nn2 = len(pos_idx), len(neg_idx)
    A2 = np2 + nn2

    # permuted, |W2|/2-scaled channel coefficient matrices (1/2 keeps the
    # fp8 hi term under the e4m3 max; sigmoid scale=2 undoes it)
    Wu2 = np.zeros((C, A2), np.float32)
    Wd2 = np.zeros((C, A2), np.float32)
    b1v = np.zeros((A2,), np.float32)
    for k, a in enumerate(list(pos_idx) + list(neg_idx)):
        g = 0.5 * abs(w2[a])
        Wu2[:, k] = g * (W1p[:, a] - W1d[:, a])
        Wd2[:, k] = g * W1d[:, a]
        b1v[k] = g * b1[a]

    u = pos @ Wu2 + b1v          # [B,T,N,A2]
    v = pos @ Wd2                # [B,T,N,A2]

    u4 = _f8_cascade(u)          # 4 x [B,T,N,A2]
    v4 = _f8_cascade(v)

    # channel-blocked column order within each t: first all positive
    # channels i-major (N*np2 cols), then all negative (N*nn2).
    # col_ch[c], col_i[c]: global channel + agent of column c.
    NA = N * A2
    col_i = np.empty(NA, np.int64)
    col_ch = np.empty(NA, np.int64)
    c = 0
    for blk_base, blk_n in ((0, np2), (np2, nn2)):
        for i in range(N):
            for a in range(blk_n):
                col_i[c] = i
                col_ch[c] = blk_base + a
                c += 1

    # DoubleRow stationary: K'=34 pairs; pair k0=(1,1)->(u1,u2),
    # k1=(1,1)->(u3,u4), 2+ch=(v1,v2)(ch), 18+ch=(v3,v4)(ch) -> (delta,delta)
    vT8 = np.zeros((B, 34, 2, T * N), dtype=f8)
    vT8[:, 0:2] = np.asarray(1.0, dtype=f8)
    for ch in range(A2):
        vT8[:, 2 + ch, 0] = v4[0][..., ch].reshape(B, T * N)
        vT8[:, 2 + ch, 1] = v4[1][..., ch].reshape(B, T * N)
        vT8[:, 18 + ch, 0] = v4[2][..., ch].reshape(B, T * N)
        vT8[:, 18 + ch, 1] = v4[3][..., ch].reshape(B, T * N)

    # moving rows: u terms per t (DMA'd each t), delta rows static
    uflat8 = np.empty((B, T, 2, 2, NA), dtype=f8)   # [b,t,krow(2),i(2),col]
    for lv in range(4):
        ulv = u4[lv].reshape(B, T, N, A2)
        uflat8[:, :, lv // 2, lv % 2] = ulv[:, :, col_i, col_ch]

    delta8 = np.zeros((32, 2, NA), dtype=f8)
    one8 = np.asarray(1.0, dtype=f8)
    for c in range(NA):
        ch = col_ch[c]
        delta8[ch, 0, c] = one8        # pairs with v1_ch
        delta8[ch, 1, c] = one8        # pairs with v2_ch
        delta8[16 + ch, 0, c] = one8   # pairs with v3_ch
        delta8[16 + ch, 1, c] = one8   # pairs with v4_ch

    # dist^2 matmul operands, fp16 hi/lo split (K=10, cross terms kept)
    f16 = np.float16
    pos64 = pos.astype(np.float64)
    n2 = (pos64 ** 2).sum(-1)        # [B,T,N] (float64)
    px = pos64[..., 0].reshape(B, T * N)
    py = pos64[..., 1].reshape(B, T * N)
    n2f = n2.reshape(B, T * N)

    def hilo(x):
        hi = x.astype(f16)
        lo = (x - hi.astype(np.float64)).astype(f16)
        return hi, lo

    pxh, pxl = hilo(px)
    pyh, pyl = hilo(py)
    n2h, n2l = hilo(n2f)
    m2pxh, m2pxl = hilo(-2 * px)
    m2pyh, m2pyl = hilo(-2 * py)
    ones = np.ones_like(pxh)
    lhsTd = np.stack([pxh, pxh, pxl, pyh, pyh, pyl, ones, ones, n2h, n2l],
                     axis=1).astype(f16)                     # [B,10,T*N]
    rhsd = np.stack([m2pxh, m2pxl, m2pxh, m2pyh, m2pyl, m2pyh, n2h, n2l,
                     ones, ones], axis=1).astype(f16)

    # final-matmul rhs, fp16 hi/lo: per t 6 cols (pxh,pyh,1, pxl,pyl,0)
    pos3 = np.empty((B, N, T * 6), f16)
    p6 = pos3.reshape(B, N, T, 6)
    p6[..., 0] = pxh.reshape(B, T, N).transpose(0, 2, 1)
    p6[..., 1] = pyh.reshape(B, T, N).transpose(0, 2, 1)
    p6[..., 2] = 1.0
    p6[..., 3] = pxl.reshape(B, T, N).transpose(0, 2, 1)
    p6[..., 4] = pyl.reshape(B, T, N).transpose(0, 2, 1)
    p6[..., 5] = 0.0

    offd1 = (1.0 - np.eye(N)).astype(f16)
    offd = np.concatenate([offd1, offd1], axis=1)   # [N, 2N]

    return dict(vT8=vT8, uflat8=uflat8, delta8=delta8, lhsTd=lhsTd,
                rhsd=rhsd, pos3=pos3, offd=offd, A2=A2, np2=np2, nn2=nn2,
                b2=float(b2[0]))


def _build_program(A2, np2, nn2, b2val):
    import concourse.bacc as bacc
    import concourse.mybir as mybir
    import concourse.tile as tile

    f32 = mybir.dt.float32
    f16 = mybir.dt.float16
    fp8 = mybir.dt.float8e4
    Alu = mybir.AluOpType
    Act = mybir.ActivationFunctionType
    X = mybir.AxisListType.X
    DR = mybir.MatmulPerfMode.DoubleRow

    NA = N * A2
    HALF = 64 * A2          # columns per PSUM half-tile

    nc = bacc.Bacc()

    vT8_p = nc.declare_dram_parameter("vT8", [34, 2, T * N], fp8, isOutput=False)
    uflat8_p = nc.declare_dram_parameter("uflat8", [T, 2, 2 * NA], fp8, isOutput=False)
    delta8_p = nc.declare_dram_parameter("delta8", [32, 2 * NA], fp8, isOutput=False)
    lhsTd_p = nc.declare_dram_parameter("lhsTd", [10, T * N], f16, isOutput=False)
    rhsd_p = nc.declare_dram_parameter("rhsd", [10, T * N], f16, isOutput=False)
    pos3_p = nc.declare_dram_parameter("pos3", [N, T * 6], f16, isOutput=False)
    offd_p = nc.declare_dram_parameter("offd", [N, 2 * N], f16, isOutput=False)
    out_p = nc.declare_dram_parameter("out", [T, N, C], f32, isOutput=True)

    with tile.TileContext(nc) as tc:
        with (
            tc.tile_pool(name="pers", bufs=1) as pers,
            tc.tile_pool(name="hpsum", bufs=2, space="PSUM") as hpsum,
            tc.tile_pool(name="dpsum", bufs=2, space="PSUM") as dpsum,
            tc.tile_pool(name="fpsum", bufs=1, space="PSUM") as fpsum,
            tc.tile_pool(name="work", bufs=3) as work,
            tc.tile_pool(name="wsmall", bufs=3) as wsmall,
        ):
            vT8_s = pers.tile([34, 2 * T * N], fp8, tag="vT8")
            lhsTd_s = pers.tile([10, T * N], f16, tag="lhsTd")
            rhsd_s = pers.tile([10, T * N], f16, tag="rhsd")
            pos3_s = pers.tile([N, T * 6], f16, tag="pos3")
            offd_s = pers.tile([N, 2 * N], f16, tag="offd")
            rhsH = [pers.tile([34, 2 * NA], fp8, tag=f"rhsH{i}", name=f"rhsH{i}")
                    for i in range(4)]

            nc.gpsimd.dma_start(vT8_s[:], vT8_p[:].rearrange("k i q -> k (i q)"))
            nc.gpsimd.dma_start(lhsTd_s[:], lhsTd_p[:])
            nc.gpsimd.dma_start(rhsd_s[:], rhsd_p[:])
            nc.gpsimd.dma_start(pos3_s[:], pos3_p[:])
            nc.gpsimd.dma_start(offd_s[:], offd_p[:])
            for i in range(4):
                nc.gpsimd.dma_start(rhsH[i][2:34, :], delta8_p[:])

            vT8v = vT8_s[:].rearrange("k (i q) -> k i q", i=2)

            # persistent PSUM bank for the final row-sum matmuls: 8 cols/t
            # cols per t: [w@pxh, w@pyh, sum_w, w@pxl, w@pyl, 0, cnt, pad]
            pf = fpsum.tile([N, 8 * T], f32, tag="F")

            chunks = []
            off = 0
            while off < HALF:
                cn = min(512, HALF - off)
                chunks.append((off, cn))
                off += cn

            mask2 = [None, None]     # the two 2t-pair masks of a 4t group
            attP2 = attM2 = w4 = pre4 = att4 = None
            pd2 = [None, None]

            for t in range(T):
                rh = rhsH[t % 4]
                nc.sync.dma_start(rh[0:2, :], uflat8_p[t])
                rhv = rh[:].rearrange("k (i f) -> k i f", i=2)

                Rt = work.tile([N, NA], f16, tag="R")
                for h in range(2):
                    ph = hpsum.tile([N, HALF], f32, tag="H")
                    for (off, cn) in chunks:
                        nc.tensor.matmul(
                            ph[:, off:off + cn],
                            vT8v[:, :, t * N:(t + 1) * N],
                            rhv[:, :, h * HALF + off:h * HALF + off + cn],
                            start=True, stop=True,
                            perf_mode=DR,
                        )
                    nc.scalar.activation(Rt[:, h * HALF:(h + 1) * HALF],
                                         ph[:], Act.Relu)

                g2 = t % 2
                g4 = t % 4
                q = g4 // 2          # which 2t-pair inside the 4t group
                if g2 == 0:
                    attP2 = wsmall.tile([N, 2 * N], f32, tag="attP")
                    attM2 = wsmall.tile([N, 2 * N], f32, tag="attM")
                    mask2[q] = wsmall.tile([N, 2 * N], f16, tag="mask", name="mask2")
                    pd2[q] = dpsum.tile([N, 2 * N], f32, tag="D", name="pd2")
                if g4 == 0:
                    pre4 = wsmall.tile([N, 4 * N], f16, tag="pre")
                    att4 = wsmall.tile([N, 4 * N], f16, tag="att")
                    w4 = wsmall.tile([N, 4 * N], f16, tag="w")

                # channel-blocked layout -> both reductions read contiguous
                # memory
                attP = attP2[:, g2 * N:(g2 + 1) * N]
                attM = attM2[:, g2 * N:(g2 + 1) * N]
                if np2 and nn2:
                    RP = Rt[:, 0:N * np2].rearrange("p (i a) -> p i a", a=np2)
                    RM = Rt[:, N * np2:NA].rearrange("p (i a) -> p i a", a=nn2)
                    nc.vector.tensor_reduce(attP, RP, axis=X, op=Alu.add)
                    nc.vector.tensor_reduce(attM, RM, axis=X, op=Alu.add)
                elif np2:
                    RP = Rt[:, 0:N * np2].rearrange("p (i a) -> p i a", a=np2)
                    nc.vector.tensor_reduce(attP, RP, axis=X, op=Alu.add)
                    nc.vector.tensor_scalar_mul(attM, attP, 0.0)
                else:
                    RM = Rt[:, N * np2:NA].rearrange("p (i a) -> p i a", a=nn2)
                    nc.vector.tensor_reduce(attM, RM, axis=X, op=Alu.add)
                    nc.vector.tensor_scalar_mul(attP, attM, 0.0)

                nc.tensor.matmul(pd2[q][:, g2 * N:(g2 + 1) * N],
                                 lhsTd_s[:, t * N:(t + 1) * N],
                                 rhsd_s[:, t * N:(t + 1) * N],
                                 start=True, stop=True)

                if g2 == 1:
                    # mask = (d^2 < R2) * offd.  The diagonal is excluded by
                    # offd; d^2 > 0 holds for every other pair (up to pairs
                    # at ~zero distance whose contribution is ~0 anyway), so
                    # the lower-bound test of the reference is not needed.
                    nc.vector.scalar_tensor_tensor(mask2[q][:], pd2[q][:], R2,
                                                   offd_s[:],
                                                   op0=Alu.is_lt, op1=Alu.mult)
                    nc.gpsimd.tensor_sub(pre4[:, (g4 - 1) * N:(g4 + 1) * N],
                                         attP2[:], attM2[:])

                if g4 == 3:
                    # scale=2 undoes the 1/2 channel prescale
                    nc.scalar.activation(att4[:], pre4[:], Act.Sigmoid,
                                         bias=b2val, scale=2.0)
                    nc.gpsimd.tensor_mul(w4[:, 0:2 * N], att4[:, 0:2 * N],
                                         mask2[0][:])
                    nc.gpsimd.tensor_mul(w4[:, 2 * N:4 * N], att4[:, 2 * N:4 * N],
                                         mask2[1][:])
                    for dt_ in range(4):
                        tt = t - 3 + dt_
                        s = dt_ * N
                        nc.tensor.matmul(pf[:, 8 * tt:8 * tt + 6],
                                         w4[:, s:s + N],
                                         pos3_s[:, 6 * tt:6 * tt + 6],
                                         start=True, stop=True)
                        nc.tensor.matmul(pf[:, 8 * tt + 6:8 * tt + 7],
                                         mask2[dt_ // 2][:, (dt_ % 2) * N:(dt_ % 2 + 1) * N],
                                         pos3_s[:, 6 * tt + 2:6 * tt + 3],
                                         start=True, stop=True)

            # ---- tail: one pass over all T timesteps ----
            pf3 = pf[:].rearrange("p (t c) -> p t c", c=8)
            p3 = pos3_s[:].rearrange("p (t c) -> p t c", c=6)
            cntT = work.tile([N, T], f32, tag="cntT")
            rcpT = work.tile([N, T], f32, tag="rcpT")
            swT = work.tile([N, 2 * T], f32, tag="swT")
            outst = work.tile([N, 2 * T], f32, tag="outst")
            nc.vector.tensor_scalar_max(cntT[:], pf3[:, :, 6], 1e-6)
            nc.vector.reciprocal(rcpT[:], cntT[:])
            s3 = swT[:].rearrange("p (c t) -> p c t", c=2)
            o3 = outst[:].rearrange("p (t c) -> p t c", c=2)
            for c in range(2):
                nc.vector.tensor_add(s3[:, c], p3[:, :, c], p3[:, :, c + 3])
                nc.vector.tensor_mul(s3[:, c], pf3[:, :, 2], s3[:, c])
                nc.vector.tensor_sub(o3[:, :, c], pf3[:, :, c], s3[:, c])
                nc.vector.tensor_add(o3[:, :, c], o3[:, :, c], pf3[:, :, c + 3])
                nc.vector.tensor_mul(o3[:, :, c], o3[:, :, c], rcpT[:])
            nc.sync.dma_start(out_p[:].rearrange("t n c -> n t c"), outst[:])

    nc.compile()
    return nc


def kernel(positions, W1, b1, W2, b2, _trace=False, _trace_kwargs=None):
    from concourse.bass_utils import run_bass_kernel_spmd

    prep = _host_prep(positions, W1, b1, W2, b2)
    A2, np2, nn2, b2v = prep["A2"], prep["np2"], prep["nn2"], prep["b2"]

    key = (A2, np2, nn2, b2v)
    if key not in _CACHE:
        _CACHE[key] = _build_program(A2, np2, nn2, b2v)
    nc = _CACHE[key]

    in_maps = []
    for b in range(B):
        in_maps.append({
            "vT8": np.ascontiguousarray(prep["vT8"][b]),
            "uflat8": np.ascontiguousarray(
                prep["uflat8"][b].reshape(T, 2, -1)),
            "delta8": prep["delta8"].reshape(32, -1),
            "lhsTd": np.ascontiguousarray(prep["lhsTd"][b]),
            "rhsd": np.ascontiguousarray(prep["rhsd"][b]),
            "pos3": np.ascontiguousarray(prep["pos3"][b]),
            "offd": prep["offd"],
        })

    kw = {}
    if _trace:
        kw["trace"] = True
        if _trace_kwargs:
            kw.update(_trace_kwargs)
    res = run_bass_kernel_spmd(nc, in_maps, list(range(B)), **kw)
    out = np.stack([r["out"] for r in res.results], axis=0).astype(np.float32)
    if _trace:
        return out, res
    return out
